# revision 1
# baseline (speedup 1.0000x reference)
"""Trainium2 Bass kernel for nn_NovaLinkPredictor (hetero GraphSAGE link predictor).

8-core SPMD strategy (edge-parallel, as per sharding hint):
  - Users sharded by range: 8 x 25088 rows (padded 200704). Movies: 8 x 10112 (padded 80896).
  - Edges bucketed by src-range (user side) and dst-sorted (movie side), on host.
  - Node degrees (means' divisors) precomputed on host from the index data.
  - Segment-sums on device with one-hot scatter matmuls (S^T @ G) accumulated in PSUM per
    4-tile super-segment (4 masked one-hots per 128-edge chunk trims gather padding);
    gathers via gpsimd.dma_gather (int16 idx, 3-range split for movie tables) — the Q7
    descriptor generation (~8ns/row) is the kernel's critical path.
  - conv1 movie-side aggregation degenerates: user_x rows are identical (u0), so
    mean = u0 * (cnt_m > 0) with host-provided indicators.
  - Tables exchanged between cores with AllGather / ReduceScatter collectives (bf16).
  - Final edge dots: labels bucketed by (user tile, movie range); user rows fetched via
    one-hot gather matmuls from usero tiles (no DMA descriptors), movie rows via dma_gather.
  - Pass B epilogue interleaved into the last range's scatter loop; movie_o root terms
    prestashed so they overlap the ReduceScatter.

The device program structure (loop bounds) is derived from max-over-core chunk counts so a
single SPMD program serves all 8 cores; per-core data (indices, one-hot keys) comes via inputs.
"""
import sys
sys.path.insert(0, "/opt/trn_rl_repo")
import numpy as np
import ml_dtypes

from concourse import bass, mybir, bacc, tile
from concourse.bass_utils import run_bass_kernel_spmd
from concourse.masks import make_identity

# ---------------- constants ----------------
H = 128
NU = 200000
NM = 80000
FD = 512
W = 8
P = 128

USR = 25088            # user rows per core (196 tiles)
UT = 196
NUP = USR * W          # 200704
MSL = 10112            # movie rows per core (79 tiles)
MT = 79
NMP = MSL * W          # 80896
GMT = NMP // P         # 632 global movie tiles

RNG_STARTS = [0, 27008, 54016]          # movie gather ranges (int16-safe)
RNG_ENDS = [27008, 54016, NMP]
NRNG = 3
GROUP = 8             # chunks per dma_gather (8*128 = 1024 rows; >1024 rows crashes)
SENT = 200.0           # one-hot sentinel (outside 0..127)

bf16 = mybir.dt.bfloat16
f32 = mybir.dt.float32
f32r = mybir.dt.float32r
i16 = mybir.dt.int16
npbf16 = ml_dtypes.bfloat16


# ---------------- host-side preprocessing ----------------

def _wrap16(idx):
    """int16 stream -> [128, n/16] wrapped layout for dma_gather idxs."""
    n = idx.shape[0]
    assert n % 16 == 0
    w = idx.reshape(n // 16, 16).T.astype(np.int16)      # [16, n/16]
    return np.ascontiguousarray(np.tile(w, (8, 1)))      # [128, n/16]


def _chunk_layout(vals, n_chunks, fill):
    """[n_chunks*128] padded stream -> [128, n_chunks] (partition-major)."""
    a = np.full(n_chunks * P, fill, dtype=vals.dtype)
    a[: len(vals)] = vals
    return np.ascontiguousarray(a.reshape(n_chunks, P).T)


def _segment_streams(gidx_list, loc_list, n_cores):
    """Given per-(core) lists of per-segment (gidx, loc) arrays keyed identically,
    pad each segment to the max-over-cores chunk count. Returns per-core
    (gidx_stream, loc_stream[128, NB]) plus per-segment chunk counts."""
    nseg = len(gidx_list[0])
    seg_chunks = []
    for s in range(nseg):
        mx = max(len(gidx_list[c][s]) for c in range(n_cores))
        seg_chunks.append((mx + P - 1) // P)
    nb = sum(seg_chunks)
    g_streams, l_streams = [], []
    for c in range(n_cores):
        g = np.zeros(nb * P, np.int16)
        l = np.full(nb * P, SENT, np.float32)
        pos = 0
        for s in range(nseg):
            n = len(gidx_list[c][s])
            g[pos: pos + n] = gidx_list[c][s]
            l[pos: pos + n] = loc_list[c][s]
            pos += seg_chunks[s] * P
        g_streams.append(g)
        l_streams.append(np.ascontiguousarray(l.reshape(nb, P).T))
    return g_streams, l_streams, seg_chunks


def preprocess(edge_src, edge_dst, lbl_user, lbl_movie):
    """Shard + sort edges/labels; build device index streams and program structure."""
    S = {}
    edge_src = np.asarray(edge_src).astype(np.int64)
    edge_dst = np.asarray(edge_dst).astype(np.int64)
    lbl_user = np.asarray(lbl_user).astype(np.int64)
    lbl_movie = np.asarray(lbl_movie).astype(np.int64)

    u_core = edge_src // USR
    u_loc = edge_src - u_core * USR

    # ---- Pass B streams: segments = (range r, 4-tile user supertile) ----
    UT4 = UT // 4
    B_g, B_k = [], []
    for c in range(W):
        m = u_core == c
        src_l = u_loc[m]
        dst = edge_dst[m]
        rng = np.minimum(dst // 27008, 2)
        t4v = src_l // (4 * P)
        order = np.lexsort((dst, t4v, rng))
        src_l, dst, rng, t4v = src_l[order], dst[order], rng[order], t4v[order]
        segs_g, segs_k = [], []
        for r in range(NRNG):
            for s in range(UT4):
                mm = (rng == r) & (t4v == s)
                sl = src_l[mm]
                tilek = (sl // P) % 4
                loc = (sl % P).astype(np.float32)
                keys = np.full((len(sl), 4), SENT, np.float32)
                keys[np.arange(len(sl)), tilek] = loc
                segs_g.append((dst[mm] - RNG_STARTS[r]).astype(np.int16))
                segs_k.append(keys)
        B_g.append(segs_g)
        B_k.append(segs_k)
    B_seg_chunks = []
    for s in range(NRNG * UT4):
        mx = max(len(B_g[c][s]) for c in range(W))
        B_seg_chunks.append((mx + P - 1) // P)
    S["B_chunks"] = np.array(B_seg_chunks).reshape(NRNG, UT4)
    S["NB"] = int(S["B_chunks"].sum())
    Bg_str, Bl_str = [], []
    for c in range(W):
        g = np.zeros(S["NB"] * P, np.int16)
        kk = np.full((S["NB"] * P, 4), SENT, np.float32)
        pos = 0
        for s in range(NRNG * UT4):
            n = len(B_g[c][s])
            g[pos: pos + n] = B_g[c][s]
            kk[pos: pos + n] = B_k[c][s]
            pos += B_seg_chunks[s] * P
        Bg_str.append(g)
        Bl_str.append(np.ascontiguousarray(
            kk.reshape(S["NB"], P, 4).transpose(1, 0, 2).reshape(P, S["NB"] * 4)))

    # ---- Pass C streams: segments = 4-tile movie supertile (dst-sorted) ----
    # Per chunk, FOUR key columns (one per tile of the supertile): key_k[slot]
    # = within-tile dst offset if the edge's tile == 4*g4+k else SENT.
    GS = GMT // 4                        # 158 supertiles
    C_g, C_k = [], []
    for c in range(W):
        m = u_core == c
        src_l = u_loc[m]
        dst = edge_dst[m]
        order = np.argsort(dst, kind="stable")
        src_l, dst = src_l[order], dst[order]
        g4 = dst // (4 * P)
        segs_g, segs_k = [], []
        for s in range(GS):
            lo = np.searchsorted(g4, s)
            hi = np.searchsorted(g4, s + 1)
            d = dst[lo:hi]
            tilek = (d // P) % 4                       # tile within supertile
            loc = (d % P).astype(np.float32)
            keys = np.full((hi - lo, 4), SENT, np.float32)
            keys[np.arange(hi - lo), tilek] = loc
            segs_g.append(src_l[lo:hi].astype(np.int16))
            segs_k.append(keys)
        C_g.append(segs_g)
        C_k.append(segs_k)
    C_seg_chunks = []
    for s in range(GS):
        mx = max(len(C_g[c][s]) for c in range(W))
        C_seg_chunks.append((mx + P - 1) // P)
    S["C_chunks"] = np.array(C_seg_chunks)          # [GS]
    S["NC"] = int(S["C_chunks"].sum())
    Cg_str, Cl_str = [], []
    for c in range(W):
        g = np.zeros(S["NC"] * P, np.int16)
        k = np.full((S["NC"] * P, 4), SENT, np.float32)
        pos = 0
        for s in range(GS):
            n = len(C_g[c][s])
            g[pos: pos + n] = C_g[c][s]
            k[pos: pos + n] = C_k[c][s]
            pos += C_seg_chunks[s] * P
        Cg_str.append(g)
        # [NC*P, 4] -> [NC, P, 4] -> [P, NC, 4] -> [P, NC*4]
        Cl_str.append(np.ascontiguousarray(
            k.reshape(S["NC"], P, 4).transpose(1, 0, 2).reshape(P, S["NC"] * 4)))

    # ---- Pass D streams: labels by user core, segments = (movie range, user tile) ----
    l_core = lbl_user // USR
    D_u, D_m, D_pos = [], [], []
    for c in range(W):
        m = l_core == c
        idxs = np.nonzero(m)[0]
        ul = (lbl_user[m] - c * USR)
        mv = lbl_movie[m]
        rng = np.minimum(mv // 27008, 2)
        tilev = ul // P
        order = np.lexsort((mv, tilev, rng))
        ul, mv, rng, tilev = ul[order], mv[order], rng[order], tilev[order]
        segs_u, segs_m = [], []
        for r in range(NRNG):
            for t in range(UT):
                mm = (rng == r) & (tilev == t)
                segs_u.append((ul[mm] - t * P).astype(np.float32))
                segs_m.append((mv[mm] - RNG_STARTS[r]).astype(np.int16))
        D_u.append(segs_u)
        D_m.append(segs_m)
        D_pos.append(idxs[order])       # original label index per real stream slot
    # pad segments to max-over-cores
    D_seg_chunks = []
    for s in range(NRNG * UT):
        mx = max(len(D_u[c][s]) for c in range(W))
        D_seg_chunks.append((mx + P - 1) // P)
    S["D_chunks"] = np.array(D_seg_chunks).reshape(NRNG, UT)
    S["ND"] = int(S["D_chunks"].sum())
    Du_str, Dm_str, D_real = [], [], []
    for c in range(W):
        du = np.full(S["ND"] * P, SENT, np.float32)
        dm = np.zeros(S["ND"] * P, np.int16)
        real = np.full(S["ND"] * P, -1, np.int64)
        pos = 0
        k = 0
        for s in range(NRNG * UT):
            n = len(D_u[c][s])
            du[pos: pos + n] = D_u[c][s]
            dm[pos: pos + n] = D_m[c][s]
            real[pos: pos + n] = D_pos[c][k: k + n]
            k += n
            pos += D_seg_chunks[s] * P
        Du_str.append(np.ascontiguousarray(du.reshape(S["ND"], P).T))
        Dm_str.append(dm)
        D_real.append(real)

    iota_rep = np.tile(np.arange(P, dtype=np.float32)[None, :], (P, 4)).astype(npbf16)

    # ---- host-side degree counts (pure index data) ----
    cnt_m_full = np.zeros(NMP, np.float32)
    cnt_m_full[:NM] = np.bincount(edge_dst, minlength=NM)[:NM]

    per_core = []
    for c in range(W):
        sl = cnt_m_full[c * MSL:(c + 1) * MSL].reshape(MT, P)
        mrecip = np.ascontiguousarray((1.0 / np.maximum(sl, 1.0)).T)
        mind = np.ascontiguousarray((sl > 0).astype(np.float32).T)
        cu = np.bincount(u_loc[u_core == c], minlength=USR).astype(np.float32)
        urecip = np.ascontiguousarray(
            (1.0 / np.maximum(cu, 1.0)).reshape(UT, P).T)
        per_core.append({
            "b_loc": Bl_str[c].astype(npbf16),
            "b_gidx": _wrap16(Bg_str[c]),
            "c_loc": Cl_str[c].astype(npbf16),
            "c_gidx": _wrap16(Cg_str[c]),
            "d_uloc": Du_str[c].astype(npbf16),
            "d_midx": _wrap16(Dm_str[c]),
            "iota": iota_rep,
            "mrecip": mrecip,
            "mind": mind,
            "urecip": urecip,
        })
    return S, per_core, D_real


def _gather_groups(n_chunks_list):
    """Split a list of per-segment chunk counts into dma_gather groups of <=GROUP chunks,
    never crossing the segment-list boundary. Returns list of group sizes (in chunks)."""
    total = sum(n_chunks_list)
    groups = []
    rem = total
    while rem > 0:
        g = min(GROUP, rem)
        groups.append(g)
        rem -= g
    return groups


# ---------------- device program ----------------

def build_program(S):
    import os
    UPTO = int(os.environ.get('KUPTO', '9'))
    nc = bacc.Bacc("TRN2", target_bir_lowering=False, debug=False, num_devices=W)
    NB, NC, ND = S["NB"], S["NC"], S["ND"]
    B_chunks, C_chunks, D_chunks = S["B_chunks"], S["C_chunks"], S["D_chunks"]

    # ---- kernel I/O ----
    featsT = nc.dram_tensor("featsT", [FD, MSL], f32, kind="ExternalInput")
    wm = nc.dram_tensor("wm", [FD, H], f32, kind="ExternalInput")
    u0 = nc.dram_tensor("u0", [H], f32, kind="ExternalInput")
    wnames = ["bm", "wl1_um", "bl1_um", "wr1_um", "wl1_mu", "bl1_mu", "wr1_mu",
              "wl2_um", "bl2_um", "wr2_um", "wl2_mu", "bl2_mu", "wr2_mu"]
    wt = {}
    for n in wnames:
        shape = [H] if n.startswith("b") else [H, H]
        wt[n] = nc.dram_tensor(n, shape, f32, kind="ExternalInput")
    iota_in = nc.dram_tensor("iota", [P, 4 * P], bf16, kind="ExternalInput")
    mrecip_in = nc.dram_tensor("mrecip", [P, MT], f32, kind="ExternalInput")
    mind_in = nc.dram_tensor("mind", [P, MT], f32, kind="ExternalInput")
    urecip_in = nc.dram_tensor("urecip", [P, UT], f32, kind="ExternalInput")
    b_loc = nc.dram_tensor("b_loc", [P, NB * 4], bf16, kind="ExternalInput")
    b_gidx = nc.dram_tensor("b_gidx", [P, NB * 8], i16, kind="ExternalInput")
    c_loc = nc.dram_tensor("c_loc", [P, NC * 4], bf16, kind="ExternalInput")
    c_gidx = nc.dram_tensor("c_gidx", [P, NC * 8], i16, kind="ExternalInput")
    d_uloc = nc.dram_tensor("d_uloc", [P, ND], bf16, kind="ExternalInput")
    d_midx = nc.dram_tensor("d_midx", [P, ND * 8], i16, kind="ExternalInput")
    out = nc.dram_tensor("out", [P, ND], f32, kind="ExternalOutput")

    # ---- internal DRAM ----
    xcat_slice = nc.dram_tensor("xcat_slice", [MSL, 2 * H], bf16)
    xcat_full = nc.dram_tensor("xcat_full", [NMP, 2 * H], bf16, addr_space="Shared")
    mht_stash = nc.dram_tensor("mht_stash", [P, MSL], bf16)
    userh = nc.dram_tensor("userh", [USR, H], bf16)
    usero = nc.dram_tensor("usero", [P, UT * H], bf16)   # tile-major: [p, t*H+h]
    partials = nc.dram_tensor("partials", [NMP, H], bf16)
    parts_rs = nc.dram_tensor("parts_rs", [MSL, H], bf16)
    mo_slice = nc.dram_tensor("mo_slice", [MSL, H], bf16)
    mo_full = nc.dram_tensor("mo_full", [NMP, H], bf16, addr_space="Shared")

    rg = [list(range(W))]

    from contextlib import ExitStack
    with tile.TileContext(nc) as tc, ExitStack() as stack:
        cst = stack.enter_context(tc.tile_pool(name="cst", bufs=1))

        # ---------- constants ----------
        iota_t = cst.tile([P, 4 * P], bf16)
        nc.sync.dma_start(out=iota_t[:], in_=iota_in[:])
        ones_bf = cst.tile([P, 1], bf16)
        nc.vector.memset(ones_bf[:], 1.0)
        ident_bf = cst.tile([P, P], bf16)
        make_identity(nc, ident_bf[:])
        ones_row = cst.tile([1, P], f32)
        nc.vector.memset(ones_row[:], 1.0)
        u0_col = cst.tile([P, 1], f32)
        nc.sync.dma_start(out=u0_col[:], in_=u0[:, None])

        wtile = {}
        for n in wnames:
            if n.startswith("b"):
                t = cst.tile([1, P], f32, tag=f"w_{n}")
                nc.sync.dma_start(out=t[:], in_=wt[n][None, :])
            else:
                t = cst.tile([P, P], f32, tag=f"w_{n}")
                nc.sync.dma_start(out=t[:], in_=wt[n][:])
            wtile[n] = t
        # casts
        w_r = {}
        for n in ["wr1_um", "wl1_mu"]:
            t = cst.tile([P, P], f32r, tag=f"wr_{n}")
            nc.vector.tensor_copy(out=t[:], in_=wtile[n][:])
            w_r[n] = t
        w_bf = {}
        for n in ["wr2_mu", "wr2_um", "wl2_um", "wl2_mu"]:
            t = cst.tile([P, P], bf16, tag=f"wbf_{n}")
            nc.vector.tensor_copy(out=t[:], in_=wtile[n][:])
            w_bf[n] = t
        wm_r = []
        for k in range(4):
            t = cst.tile([P, H], f32r, tag=f"wm_{k}")
            nc.sync.dma_start(out=t[:], in_=wm[k * P:(k + 1) * P, :].bitcast(f32r))
            wm_r.append(t)
        bm_col = cst.tile([P, 1], f32)
        nc.sync.dma_start(out=bm_col[:], in_=wt["bm"][:, None])
        bl1um_col = cst.tile([P, 1], f32)
        nc.sync.dma_start(out=bl1um_col[:], in_=wt["bl1_um"][:, None])

        # v_row = u0 @ Wl1_um   [1,128]; r1_row = u0 @ Wr1_mu
        with tc.tile_pool(name="psc", bufs=2, space="PSUM") as psc:
            vp = psc.tile([1, P], f32, space="PSUM", tag="vrow")
            nc.tensor.matmul(out=vp[:], lhsT=u0_col[:], rhs=wtile["wl1_um"][:], start=True, stop=True)
            v_row = cst.tile([1, P], f32)
            nc.vector.tensor_copy(out=v_row[:], in_=vp[:])
            rp = psc.tile([1, P], f32, space="PSUM", tag="vrow")
            nc.tensor.matmul(out=rp[:], lhsT=u0_col[:], rhs=wtile["wr1_mu"][:], start=True, stop=True)
            b1_row = cst.tile([1, P], f32)
            nc.vector.tensor_tensor(out=b1_row[:], in0=rp[:], in1=wtile["bl1_mu"][:], op=mybir.AluOpType.add)

            def bcast_row(row_ap, tag):
                ps = psc.tile([P, P], f32, space="PSUM", tag="bcast")
                nc.tensor.matmul(out=ps[:], lhsT=ones_row[:], rhs=row_ap, start=True, stop=True)
                t = cst.tile([P, P], f32, tag=tag)
                nc.vector.tensor_copy(out=t[:], in_=ps[:])
                return t

            Vcast = bcast_row(v_row[:], "Vcast")
            B1cast = bcast_row(b1_row[:], "B1cast")
            B2cast = bcast_row(wtile["bl2_mu"][:], "B2cast")
            B3cast = bcast_row(wtile["bl2_um"][:], "B3cast")

        if UPTO >= 2:
            # ---------- Stage 0: movie-side tables ----------
            NCT = (MSL + 511) // 512     # 20 col-tiles (last = 384)
            with tc.tile_pool(name="s0_sb", bufs=1) as s0_sb, \
                 tc.tile_pool(name="s0_mx", bufs=2) as s0_mx, \
                 tc.tile_pool(name="s0_ft", bufs=3) as s0_ft, \
                 tc.tile_pool(name="s0_ps", bufs=1, space="PSUM") as s0_ps, \
                 tc.tile_pool(name="s0_pt", bufs=2, space="PSUM") as s0_pt, \
                 tc.tile_pool(name="s0_stg", bufs=3) as s0_stg:
                p1T = s0_sb.tile([P, MSL], bf16)
                A_fm = s0_sb.tile([P, MSL], bf16)
                mhT = s0_sb.tile([P, MSL], bf16)
                indcols = s0_sb.tile([P, MT], f32)
                nc.sync.dma_start(out=indcols[:], in_=mind_in[:])

                for j in range(NCT):
                    c0 = j * 512
                    cw = min(512, MSL - c0)
                    mxps = s0_ps.tile([P, 512], f32, space="PSUM", tag="mx")
                    for k in range(4):
                        ft = s0_ft.tile([P, 512], f32r, tag="ft")
                        nc.sync.dma_start(out=ft[:, :cw], in_=featsT[k * P:(k + 1) * P, c0:c0 + cw].bitcast(f32r))
                        nc.tensor.matmul(out=mxps[:, :cw], lhsT=wm_r[k][:], rhs=ft[:, :cw],
                                         start=(k == 0), stop=(k == 3))
                    mxt = s0_mx.tile([P, 512], f32r, tag="mxt")
                    nc.vector.tensor_tensor(out=mxt[:, :cw], in0=mxps[:, :cw],
                                            in1=bm_col[:].to_broadcast([P, cw]),
                                            op=mybir.AluOpType.add)
                    p1ps = s0_ps.tile([P, 512], f32, space="PSUM", tag="p1")
                    nc.tensor.matmul(out=p1ps[:, :cw], lhsT=w_r["wl1_mu"][:], rhs=mxt[:, :cw],
                                     start=True, stop=True)
                    nc.scalar.copy(out=p1T[:, c0:c0 + cw], in_=p1ps[:, :cw])
                    aps = s0_ps.tile([P, 512], f32, space="PSUM", tag="A")
                    nc.tensor.matmul(out=aps[:, :cw], lhsT=w_r["wr1_um"][:], rhs=mxt[:, :cw],
                                     start=True, stop=True)
                    nc.vector.tensor_tensor(out=A_fm[:, c0:c0 + cw], in0=aps[:, :cw],
                                            in1=bl1um_col[:].to_broadcast([P, cw]),
                                            op=mybir.AluOpType.add)

                # per 128-tile: movie_h row-major then back to feature-major
                for t in range(MT):
                    c0 = t * P
                    tp = s0_pt.tile([P, P], bf16, space="PSUM", tag="tp")
                    nc.tensor.transpose(out=tp[:], in_=A_fm[:, c0:c0 + P], identity=ident_bf[:])
                    term = s0_stg.tile([P, P], f32, tag="term")
                    nc.vector.tensor_tensor(out=term[:], in0=Vcast[:],
                                            in1=indcols[:, t:t + 1].to_broadcast([P, P]),
                                            op=mybir.AluOpType.mult)
                    mhrow = s0_stg.tile([P, P], bf16, tag="mhrow")
                    nc.vector.tensor_tensor(out=mhrow[:], in0=tp[:], in1=term[:],
                                            op=mybir.AluOpType.add)
                    nc.vector.tensor_scalar_max(out=mhrow[:], in0=mhrow[:], scalar1=0.0)
                    tp2 = s0_pt.tile([P, P], bf16, space="PSUM", tag="tp2")
                    nc.tensor.transpose(out=tp2[:], in_=mhrow[:], identity=ident_bf[:])
                    nc.scalar.copy(out=mhT[:, c0:c0 + P], in_=tp2[:])
                nc.sync.dma_start(out=mht_stash[:], in_=mhT[:])

                # p2T = Wl2_mu.T @ mhT  (bf16)
                p2T = s0_sb.tile([P, MSL], bf16)
                for j in range(NCT):
                    c0 = j * 512
                    cw = min(512, MSL - c0)
                    ps = s0_ps.tile([P, 512], f32, space="PSUM", tag="p2")
                    nc.tensor.matmul(out=ps[:, :cw], lhsT=w_bf["wl2_mu"][:], rhs=mhT[:, c0:c0 + cw],
                                     start=True, stop=True)
                    nc.scalar.copy(out=p2T[:, c0:c0 + cw], in_=ps[:, :cw])

                # transpose to row-major X_cat slice and store
                for t in range(MT):
                    c0 = t * P
                    stg = s0_stg.tile([P, 2 * H], bf16, tag="xrow")
                    tp = s0_pt.tile([P, P], bf16, space="PSUM", tag="tp")
                    nc.tensor.transpose(out=tp[:], in_=p1T[:, c0:c0 + P], identity=ident_bf[:])
                    nc.scalar.copy(out=stg[:, 0:H], in_=tp[:])
                    tp2 = s0_pt.tile([P, P], bf16, space="PSUM", tag="tp2")
                    nc.tensor.transpose(out=tp2[:], in_=p2T[:, c0:c0 + P], identity=ident_bf[:])
                    nc.scalar.copy(out=stg[:, H:2 * H], in_=tp2[:])
                    nc.sync.dma_start(out=xcat_slice[c0:c0 + P, :], in_=stg[:])

            nc.gpsimd.collective_compute(
                "AllGather", mybir.AluOpType.bypass, replica_groups=rg,
                ins=[xcat_slice[:].opt()], outs=[xcat_full[:].opt()])

        if UPTO >= 3:
            # ---------- Pass B: user-side fused aggregation ----------
            ACC = 256  # [p1sum 128 | p2sum 128]; counts come from host
            with tc.tile_pool(name="pb_sb", bufs=1) as pb_sb, \
                 tc.tile_pool(name="pb_s4", bufs=8) as pb_s4, \
                 tc.tile_pool(name="pb_g", bufs=5) as pb_g, \
                 tc.tile_pool(name="pb_gi", bufs=5) as pb_gi, \
                 tc.tile_pool(name="pb_ps", bufs=1, space="PSUM") as pb_ps, \
                 tc.tile_pool(name="pb_pt", bufs=3, space="PSUM") as pb_pt, \
                 tc.tile_pool(name="pb_stg", bufs=8) as pb_stg:
                bloc_t = pb_sb.tile([P, NB * 4], bf16)
                nc.sync.dma_start(out=bloc_t[:], in_=b_loc[:])
                accB = pb_sb.tile([P, UT * ACC], bf16)
                nc.vector.memset(accB[:], 0.0)
                recipv = pb_sb.tile([P, UT], f32)
                nc.sync.dma_start(out=recipv[:], in_=urecip_in[:])

                def b_epilogue(t):
                    # user_h / user_o for tile t (accB final after last range)
                    a0 = t * ACC
                    rc = recipv[:, t:t + 1]
                    uh = pb_stg.tile([P, H], bf16, tag="uh")
                    nc.vector.tensor_tensor(out=uh[:], in0=accB[:, a0:a0 + H],
                                            in1=rc.to_broadcast([P, H]), op=mybir.AluOpType.mult)
                    nc.vector.tensor_tensor(out=uh[:], in0=uh[:], in1=B1cast[:],
                                            op=mybir.AluOpType.add)
                    nc.vector.tensor_scalar_max(out=uh[:], in0=uh[:], scalar1=0.0)
                    tp = pb_pt.tile([P, P], bf16, space="PSUM", tag="ep")
                    nc.tensor.transpose(out=tp[:], in_=uh[:], identity=ident_bf[:])
                    uht = pb_stg.tile([P, P], bf16, tag="uhts")
                    nc.scalar.copy(out=uht[:], in_=tp[:])
                    # Pass C gather table = user_h @ Wl2_um (pre-multiplied; linearity)
                    t2ps = pb_pt.tile([P, P], f32, space="PSUM", tag="ep")
                    nc.tensor.matmul(out=t2ps[:], lhsT=uht[:], rhs=w_bf["wl2_um"][:],
                                     start=True, stop=True)
                    uh2 = pb_stg.tile([P, P], bf16, tag="uh2")
                    nc.scalar.copy(out=uh2[:], in_=t2ps[:])
                    nc.sync.dma_start(out=userh[t * P:(t + 1) * P, :], in_=uh2[:])
                    # user_o = p2sum*recip + B2cast + uh @ Wr2_mu
                    rps = pb_pt.tile([P, P], f32, space="PSUM", tag="ep")
                    nc.tensor.matmul(out=rps[:], lhsT=uht[:], rhs=w_bf["wr2_mu"][:],
                                     start=True, stop=True)
                    uo = pb_stg.tile([P, H], f32, tag="uo")
                    nc.vector.tensor_tensor(out=uo[:], in0=accB[:, a0 + H:a0 + 2 * H],
                                            in1=rc.to_broadcast([P, H]), op=mybir.AluOpType.mult)
                    nc.vector.tensor_tensor(out=uo[:], in0=uo[:], in1=B2cast[:],
                                            op=mybir.AluOpType.add)
                    uo_bf = pb_stg.tile([P, H], bf16, tag="uobf")
                    nc.vector.tensor_tensor(out=uo_bf[:], in0=uo[:], in1=rps[:],
                                            op=mybir.AluOpType.add)
                    nc.sync.dma_start(out=usero[:, t * H:(t + 1) * H], in_=uo_bf[:])

                pos = 0          # global chunk position (stream)
                for r in range(NRNG):
                    table = xcat_full[RNG_STARTS[r]:RNG_ENDS[r], :]
                    sub_chunks = int(B_chunks[r].sum())
                    # gather groups for this sub-pass
                    gpos = 0
                    gbufs = []
                    while gpos < sub_chunks:
                        gn = min(GROUP, sub_chunks - gpos)
                        gb = pb_g.tile([P, GROUP * 2 * H], bf16, tag="gbuf")
                        gi = pb_gi.tile([P, GROUP * 8], i16, tag="gidx")
                        col0 = (pos + gpos) * 8
                        nc.sync.dma_start(out=gi[:, :gn * 8], in_=b_gidx[:, col0: col0 + gn * 8])
                        nc.gpsimd.dma_gather(
                            out_ap=gb[:, :gn * 2 * H].rearrange("p (c n) -> p c n", c=gn),
                            in_ap=table,
                            idxs_ap=gi[:, :gn * 8],
                            num_idxs=gn * P,
                            num_idxs_reg=gn * P,
                            elem_size=2 * H,
                        )
                        gbufs.append((gpos, gn, gb))
                        gpos += gn

                    def get_slot(sub_pos):
                        for g0, gn, gb in gbufs:
                            if g0 <= sub_pos < g0 + gn:
                                return gb, sub_pos - g0
                        raise AssertionError

                    sub_pos = 0
                    for t4 in range(UT // 4):
                        n = int(B_chunks[r][t4])
                        if n == 0:
                            if r == NRNG - 1:
                                for k in range(4):
                                    b_epilogue(4 * t4 + k)
                            continue
                        ps4 = []
                        for k in range(4):
                            pst = pb_ps.tile([P, 2 * H], f32, space="PSUM", tag=f"bps{k}")
                            ps4.append(pst[:])
                        for ch in range(n):
                            cc = pos + sub_pos + ch
                            s4 = pb_s4.tile([P, 4 * P], bf16, tag="s4")
                            nc.vector.tensor_tensor(
                                out=s4[:].rearrange("p (k n) -> p k n", k=4),
                                in0=iota_t[:].rearrange("p (k n) -> p k n", k=4),
                                in1=bloc_t[:, 4 * cc: 4 * cc + 4][:, :, None].to_broadcast([P, 4, P]),
                                op=mybir.AluOpType.is_equal,
                            )
                            gb, slot = get_slot(sub_pos + ch)
                            for k in range(4):
                                nc.tensor.matmul(
                                    out=ps4[k],
                                    lhsT=s4[:, k * P:(k + 1) * P],
                                    rhs=gb[:, slot * 2 * H:(slot + 1) * 2 * H],
                                    start=(ch == 0), stop=(ch == n - 1),
                                )
                        for k in range(4):
                            t = 4 * t4 + k
                            a0 = t * ACC
                            nc.vector.tensor_tensor(out=accB[:, a0:a0 + 2 * H], in0=ps4[k],
                                                    in1=accB[:, a0:a0 + 2 * H], op=mybir.AluOpType.add)
                            if r == NRNG - 1:
                                b_epilogue(t)
                        sub_pos += n
                    pos += sub_chunks

        if UPTO >= 4:
            # ---------- Pass C: movie-side aggregation of user_h ----------
            GS = GMT // 4
            with tc.tile_pool(name="pc_sb", bufs=1) as pc_sb, \
                 tc.tile_pool(name="pc_s4", bufs=8) as pc_s4, \
                 tc.tile_pool(name="pc_g", bufs=5) as pc_g, \
                 tc.tile_pool(name="pc_gi", bufs=5) as pc_gi, \
                 tc.tile_pool(name="pc_ps", bufs=2, space="PSUM") as pc_ps, \
                 tc.tile_pool(name="pc_stg", bufs=8) as pc_stg:
                cloc_t = pc_sb.tile([P, NC * 4], bf16)
                nc.sync.dma_start(out=cloc_t[:], in_=c_loc[:])

                gpos = 0
                gbufs = []
                while gpos < NC:
                    gn = min(GROUP, NC - gpos)
                    gb = pc_g.tile([P, GROUP * H], bf16, tag="gbuf")
                    gi = pc_gi.tile([P, GROUP * 8], i16, tag="gidx")
                    nc.sync.dma_start(out=gi[:, :gn * 8], in_=c_gidx[:, gpos * 8: (gpos + gn) * 8])
                    nc.gpsimd.dma_gather(
                        out_ap=gb[:, :gn * H].rearrange("p (c n) -> p c n", c=gn),
                        in_ap=userh[:],
                        idxs_ap=gi[:, :gn * 8],
                        num_idxs=gn * P,
                        num_idxs_reg=gn * P,
                        elem_size=H,
                    )
                    gbufs.append((gpos, gn, gb))
                    gpos += gn

                def get_slotC(p_):
                    for g0, gn, gb in gbufs:
                        if g0 <= p_ < g0 + gn:
                            return gb, p_ - g0
                    raise AssertionError

                pos = 0
                for s in range(GS):
                    n = int(C_chunks[s])
                    if n == 0:
                        for k in range(4):
                            stg = pc_stg.tile([P, H], bf16, tag="pstg")
                            nc.vector.memset(stg[:], 0.0)
                            g = 4 * s + k
                            nc.sync.dma_start(out=partials[g * P:(g + 1) * P, :], in_=stg[:])
                        continue
                    ps4 = []
                    for k in range(4):
                        pst = pc_ps.tile([P, H], f32, space="PSUM", tag=f"ps{k}")
                        ps4.append(pst[:])
                    for ch in range(n):
                        cc = pos + ch
                        s4 = pc_s4.tile([P, 4 * P], bf16, tag="s4")
                        nc.vector.tensor_tensor(
                            out=s4[:].rearrange("p (k n) -> p k n", k=4),
                            in0=iota_t[:].rearrange("p (k n) -> p k n", k=4),
                            in1=cloc_t[:, 4 * cc: 4 * cc + 4][:, :, None].to_broadcast([P, 4, P]),
                            op=mybir.AluOpType.is_equal,
                        )
                        gb, slot = get_slotC(cc)
                        for k in range(4):
                            nc.tensor.matmul(
                                out=ps4[k],
                                lhsT=s4[:, k * P:(k + 1) * P],
                                rhs=gb[:, slot * H:(slot + 1) * H],
                                start=(ch == 0), stop=(ch == n - 1),
                            )
                    for k in range(4):
                        stg = pc_stg.tile([P, H], bf16, tag="pstg")
                        nc.scalar.copy(out=stg[:], in_=ps4[k])
                        g = 4 * s + k
                        nc.sync.dma_start(out=partials[g * P:(g + 1) * P, :], in_=stg[:])
                    pos += n

        if UPTO >= 5:
            # ---------- movie_o (root terms prestashed to overlap the RS) ----------
            with tc.tile_pool(name="mo_sb", bufs=1) as mo_sb, \
                 tc.tile_pool(name="mo_in", bufs=6) as mo_in, \
                 tc.tile_pool(name="mo_ps", bufs=2, space="PSUM") as mo_ps, \
                 tc.tile_pool(name="mo_stg", bufs=4) as mo_stg:
                recipm = mo_sb.tile([P, MT], f32)
                nc.sync.dma_start(out=recipm[:], in_=mrecip_in[:])
                roots = mo_sb.tile([P, MT * H], bf16)
                for t in range(MT):
                    mh = mo_in.tile([P, P], bf16, tag="mh")
                    nc.sync.dma_start(out=mh[:], in_=mht_stash[:, t * P:(t + 1) * P])
                    rps = mo_ps.tile([P, P], f32, space="PSUM", tag="mroot")
                    nc.tensor.matmul(out=rps[:], lhsT=mh[:], rhs=w_bf["wr2_um"][:],
                                     start=True, stop=True)
                    nc.vector.tensor_tensor(out=roots[:, t * H:(t + 1) * H], in0=rps[:],
                                            in1=B3cast[:], op=mybir.AluOpType.add)

                nc.gpsimd.collective_compute(
                    "ReduceScatter", mybir.AluOpType.add, replica_groups=rg,
                    ins=[partials[:].opt()], outs=[parts_rs[:].opt()])

                for t in range(MT):
                    pin = mo_in.tile([P, H], bf16, tag="pin")
                    nc.sync.dma_start(out=pin[:], in_=parts_rs[t * P:(t + 1) * P, :])
                    mo_t = mo_stg.tile([P, H], f32, tag="mo1")
                    nc.vector.tensor_tensor(out=mo_t[:], in0=pin[:],
                                            in1=recipm[:, t:t + 1].to_broadcast([P, H]),
                                            op=mybir.AluOpType.mult)
                    mo_bf = mo_stg.tile([P, H], bf16, tag="mo2")
                    nc.vector.tensor_tensor(out=mo_bf[:], in0=mo_t[:],
                                            in1=roots[:, t * H:(t + 1) * H],
                                            op=mybir.AluOpType.add)
                    nc.sync.dma_start(out=mo_slice[t * P:(t + 1) * P, :], in_=mo_bf[:])

            nc.gpsimd.collective_compute(
                "AllGather", mybir.AluOpType.bypass, replica_groups=rg,
                ins=[mo_slice[:].opt()], outs=[mo_full[:].opt()])

        if UPTO >= 6:
            # ---------- Pass D: label dots (user side via one-hot gather) ----------
            with tc.tile_pool(name="pd_sb", bufs=1) as pd_sb, \
                 tc.tile_pool(name="pd_g", bufs=8) as pd_g, \
                 tc.tile_pool(name="pd_gi", bufs=8) as pd_gi, \
                 tc.tile_pool(name="pd_pt", bufs=4, space="PSUM") as pd_pt, \
                 tc.tile_pool(name="pd_stg", bufs=8) as pd_stg:
                outstrip = pd_sb.tile([P, ND], f32)
                dloc_t = pd_sb.tile([P, ND], bf16)
                nc.sync.dma_start(out=dloc_t[:], in_=d_uloc[:])
                uo_all = pd_sb.tile([P, UT * H], bf16)
                nc.sync.dma_start(out=uo_all[:], in_=usero[:])
                pos = 0
                for r in range(NRNG):
                    sub_chunks = int(D_chunks[r].sum())
                    table = mo_full[RNG_STARTS[r]:RNG_ENDS[r], :]
                    gpos = 0
                    gbufs = []
                    while gpos < sub_chunks:
                        gn = min(GROUP, sub_chunks - gpos)
                        gm = pd_g.tile([P, GROUP * H], bf16, tag="gm")
                        gim = pd_gi.tile([P, GROUP * 8], i16, tag="gim")
                        col0 = (pos + gpos) * 8
                        nc.sync.dma_start(out=gim[:, :gn * 8], in_=d_midx[:, col0: col0 + gn * 8])
                        nc.gpsimd.dma_gather(
                            out_ap=gm[:, :gn * H].rearrange("p (c n) -> p c n", c=gn),
                            in_ap=table, idxs_ap=gim[:, :gn * 8],
                            num_idxs=gn * P, num_idxs_reg=gn * P, elem_size=H)
                        gbufs.append((gpos, gn, gm))
                        gpos += gn

                    def get_slotD(p_, _gb=gbufs):
                        for g0, gn, gb in _gb:
                            if g0 <= p_ < g0 + gn:
                                return gb, p_ - g0
                        raise AssertionError

                    sub_pos = 0
                    for t in range(UT):
                        n = int(D_chunks[r][t])
                        if n == 0:
                            continue
                        uo_t = uo_all[:, t * H:(t + 1) * H]
                        for ch in range(n):
                            cc = pos + sub_pos + ch
                            s1 = pd_stg.tile([P, P], bf16, tag="s1")
                            nc.vector.tensor_tensor(
                                out=s1[:], in0=iota_t[:, 0:P],
                                in1=dloc_t[:, cc:cc + 1].to_broadcast([P, P]),
                                op=mybir.AluOpType.is_equal)
                            tp = pd_pt.tile([P, P], bf16, space="PSUM", tag="tp")
                            nc.tensor.transpose(out=tp[:], in_=s1[:], identity=ident_bf[:])
                            oh = pd_stg.tile([P, P], bf16, tag="oh")
                            nc.scalar.copy(out=oh[:], in_=tp[:])
                            ups = pd_pt.tile([P, H], f32, space="PSUM", tag="ups")
                            nc.tensor.matmul(out=ups[:], lhsT=oh[:], rhs=uo_t,
                                             start=True, stop=True)
                            gb, slot = get_slotD(sub_pos + ch)
                            pr = pd_stg.tile([P, H], f32, tag="pr")
                            nc.vector.tensor_tensor(out=pr[:], in0=ups[:],
                                                    in1=gb[:, slot * H:(slot + 1) * H],
                                                    op=mybir.AluOpType.mult)
                            nc.vector.tensor_reduce(
                                out=outstrip[:, cc:cc + 1], in_=pr[:],
                                axis=mybir.AxisListType.X, op=mybir.AluOpType.add)
                        sub_pos += n
                    pos += sub_chunks
                nc.sync.dma_start(out=out[:], in_=outstrip[:])
        else:
            with tc.tile_pool(name="dummy", bufs=1) as dp:
                z = dp.tile([P, ND], f32)
                nc.vector.memset(z[:], 0.0)
                nc.sync.dma_start(out=out[:], in_=z[:])

    nc.compile()
    return nc


# ---------------- entry point ----------------

_CACHE = {}
TRACE = False
LAST_EXEC_NS = None
LAST_RESULTS = None


def kernel(movie_feats, user_init, edge_src, edge_dst, lbl_user, lbl_movie, n_users,
           Wm, bm,
           Wl1_um, bl1_um, Wr1_um, Wl1_mu, bl1_mu, Wr1_mu,
           Wl2_um, bl2_um, Wr2_um, Wl2_mu, bl2_mu, Wr2_mu):
    movie_feats = np.asarray(movie_feats, dtype=np.float32)
    S, per_core, D_real = preprocess(edge_src, edge_dst, lbl_user, lbl_movie)

    key = (S["NB"], S["NC"], S["ND"],
           S["B_chunks"].tobytes(), S["C_chunks"].tobytes(), S["D_chunks"].tobytes())
    if key in _CACHE:
        nc = _CACHE[key]
    else:
        nc = build_program(S)
        _CACHE[key] = nc

    featsT = np.zeros((FD, NMP), np.float32)
    featsT[:, :NM] = movie_feats.T

    weights = {
        "wm": np.asarray(Wm, np.float32), "u0": np.asarray(user_init, np.float32),
        "bm": np.asarray(bm, np.float32),
        "wl1_um": np.asarray(Wl1_um, np.float32), "bl1_um": np.asarray(bl1_um, np.float32),
        "wr1_um": np.asarray(Wr1_um, np.float32),
        "wl1_mu": np.asarray(Wl1_mu, np.float32), "bl1_mu": np.asarray(bl1_mu, np.float32),
        "wr1_mu": np.asarray(Wr1_mu, np.float32),
        "wl2_um": np.asarray(Wl2_um, np.float32), "bl2_um": np.asarray(bl2_um, np.float32),
        "wr2_um": np.asarray(Wr2_um, np.float32),
        "wl2_mu": np.asarray(Wl2_mu, np.float32), "bl2_mu": np.asarray(bl2_mu, np.float32),
        "wr2_mu": np.asarray(Wr2_mu, np.float32),
    }

    in_maps = []
    for c in range(W):
        m = {"featsT": np.ascontiguousarray(featsT[:, c * MSL:(c + 1) * MSL])}
        m.update(weights)
        pc = per_core[c]
        m.update({
            "iota": pc["iota"],
            "b_loc": pc["b_loc"], "b_gidx": pc["b_gidx"],
            "c_loc": pc["c_loc"], "c_gidx": pc["c_gidx"],
            "d_uloc": pc["d_uloc"], "d_midx": pc["d_midx"],
            "mrecip": pc["mrecip"], "mind": pc["mind"], "urecip": pc["urecip"],
        })
        in_maps.append(m)

    global LAST_EXEC_NS, LAST_RESULTS
    res = run_bass_kernel_spmd(nc, in_maps, core_ids=list(range(W)), trace=TRACE)
    LAST_EXEC_NS = res.exec_time_ns
    LAST_RESULTS = res

    EL = len(np.asarray(lbl_user))
    out_full = np.zeros(EL, np.float32)
    for c in range(W):
        vals = res.results[c]["out"].T.reshape(-1)       # stream order
        real = D_real[c]
        mask = real >= 0
        out_full[real[mask]] = vals[mask]
    return out_full



# revision 3
# speedup vs baseline: 1.1779x; 1.1779x over previous
"""Trainium2 Bass kernel for nn_NovaLinkPredictor (hetero GraphSAGE link predictor).

8-core SPMD strategy (edge-parallel, as per sharding hint):
  - Users sharded by range: 8 x 25088 rows (padded 200704). Movies: 8 x 10112 (padded 80896).
  - Edges bucketed by src-range (user side) and dst-sorted (movie side), on host.
  - Node degrees (means' divisors) precomputed on host from the index data.
  - Segment-sums on device with one-hot scatter matmuls (S^T @ G) accumulated in PSUM per
    4-tile super-segment (4 masked one-hots per 128-edge chunk trims gather padding);
    gathers via gpsimd.dma_gather (int16 idx, 3-range split for movie tables) — the Q7
    descriptor generation (~8ns/row) is the kernel's critical path.
  - conv1 movie-side aggregation degenerates: user_x rows are identical (u0), so
    mean = u0 * (cnt_m > 0) with host-provided indicators.
  - Tables exchanged between cores with AllGather / ReduceScatter collectives (bf16).
  - Final edge dots: labels bucketed by (user tile, movie range); user rows fetched via
    one-hot gather matmuls from usero tiles (no DMA descriptors), movie rows via dma_gather.
  - Pass B epilogue interleaved into the last range's scatter loop; movie_o root terms
    prestashed so they overlap the ReduceScatter.

The device program structure (loop bounds) is derived from max-over-core chunk counts so a
single SPMD program serves all 8 cores; per-core data (indices, one-hot keys) comes via inputs.
"""
import sys
sys.path.insert(0, "/opt/trn_rl_repo")
import numpy as np
import ml_dtypes

from concourse import bass, mybir, bacc, tile
from concourse.bass_utils import run_bass_kernel_spmd
from concourse.masks import make_identity

# ---------------- constants ----------------
H = 128
NU = 200000
NM = 80000
FD = 512
W = 8
P = 128

USR = 25088            # user rows per core (196 tiles)
UT = 196
NUP = USR * W          # 200704
MSL = 10112            # movie rows per core (79 tiles)
MT = 79
NMP = MSL * W          # 80896
GMT = NMP // P         # 632 global movie tiles

RNG_STARTS = [0, 27008, 54016]          # movie gather ranges (int16-safe)
RNG_ENDS = [27008, 54016, NMP]
NRNG = 3
GROUP = 8             # chunks per dma_gather (8*128 = 1024 rows; >1024 rows crashes)
SENT = 200.0           # one-hot sentinel (outside 0..127)

bf16 = mybir.dt.bfloat16
f32 = mybir.dt.float32
f32r = mybir.dt.float32r
i16 = mybir.dt.int16
npbf16 = ml_dtypes.bfloat16


# ---------------- host-side preprocessing ----------------

def _wrap16(idx):
    """int16 stream -> [128, n/16] wrapped layout for dma_gather idxs."""
    n = idx.shape[0]
    assert n % 16 == 0
    w = idx.reshape(n // 16, 16).T.astype(np.int16)      # [16, n/16]
    return np.ascontiguousarray(np.tile(w, (8, 1)))      # [128, n/16]


def _chunk_layout(vals, n_chunks, fill):
    """[n_chunks*128] padded stream -> [128, n_chunks] (partition-major)."""
    a = np.full(n_chunks * P, fill, dtype=vals.dtype)
    a[: len(vals)] = vals
    return np.ascontiguousarray(a.reshape(n_chunks, P).T)


def _segment_streams(gidx_list, loc_list, n_cores):
    """Given per-(core) lists of per-segment (gidx, loc) arrays keyed identically,
    pad each segment to the max-over-cores chunk count. Returns per-core
    (gidx_stream, loc_stream[128, NB]) plus per-segment chunk counts."""
    nseg = len(gidx_list[0])
    seg_chunks = []
    for s in range(nseg):
        mx = max(len(gidx_list[c][s]) for c in range(n_cores))
        seg_chunks.append((mx + P - 1) // P)
    nb = sum(seg_chunks)
    g_streams, l_streams = [], []
    for c in range(n_cores):
        g = np.zeros(nb * P, np.int16)
        l = np.full(nb * P, SENT, np.float32)
        pos = 0
        for s in range(nseg):
            n = len(gidx_list[c][s])
            g[pos: pos + n] = gidx_list[c][s]
            l[pos: pos + n] = loc_list[c][s]
            pos += seg_chunks[s] * P
        g_streams.append(g)
        l_streams.append(np.ascontiguousarray(l.reshape(nb, P).T))
    return g_streams, l_streams, seg_chunks


def preprocess(edge_src, edge_dst, lbl_user, lbl_movie):
    """Shard + sort edges/labels; build device index streams and program structure."""
    S = {}
    edge_src = np.asarray(edge_src).astype(np.int64)
    edge_dst = np.asarray(edge_dst).astype(np.int64)
    lbl_user = np.asarray(lbl_user).astype(np.int64)
    lbl_movie = np.asarray(lbl_movie).astype(np.int64)

    u_core = edge_src // USR
    u_loc = edge_src - u_core * USR

    # ---- Pass B streams: segments = (range r, 4-tile user supertile) ----
    UT4 = UT // 4
    B_g, B_k = [], []
    for c in range(W):
        m = u_core == c
        src_l = u_loc[m]
        dst = edge_dst[m]
        rng = np.minimum(dst // 27008, 2)
        t4v = src_l // (4 * P)
        order = np.lexsort((dst, t4v, rng))
        src_l, dst, rng, t4v = src_l[order], dst[order], rng[order], t4v[order]
        segs_g, segs_k = [], []
        for r in range(NRNG):
            for s in range(UT4):
                mm = (rng == r) & (t4v == s)
                sl = src_l[mm]
                tilek = (sl // P) % 4
                loc = (sl % P).astype(np.float32)
                keys = np.full((len(sl), 4), SENT, np.float32)
                keys[np.arange(len(sl)), tilek] = loc
                segs_g.append((dst[mm] - RNG_STARTS[r]).astype(np.int16))
                segs_k.append(keys)
        B_g.append(segs_g)
        B_k.append(segs_k)
    B_seg_chunks = []
    for s in range(NRNG * UT4):
        mx = max(len(B_g[c][s]) for c in range(W))
        B_seg_chunks.append((mx + P - 1) // P)
    S["B_chunks"] = np.array(B_seg_chunks).reshape(NRNG, UT4)
    S["NB"] = int(S["B_chunks"].sum())
    Bg_str, Bl_str = [], []
    for c in range(W):
        g = np.zeros(S["NB"] * P, np.int16)
        kk = np.full((S["NB"] * P, 4), SENT, np.float32)
        pos = 0
        for s in range(NRNG * UT4):
            n = len(B_g[c][s])
            g[pos: pos + n] = B_g[c][s]
            kk[pos: pos + n] = B_k[c][s]
            pos += B_seg_chunks[s] * P
        Bg_str.append(g)
        Bl_str.append(np.ascontiguousarray(
            kk.reshape(S["NB"], P, 4).transpose(1, 0, 2).reshape(P, S["NB"] * 4)))

    # ---- Pass C streams: segments = 4-tile movie supertile (dst-sorted) ----
    # Per chunk, FOUR key columns (one per tile of the supertile): key_k[slot]
    # = within-tile dst offset if the edge's tile == 4*g4+k else SENT.
    GS = GMT // 4                        # 158 supertiles
    C_g, C_k = [], []
    for c in range(W):
        m = u_core == c
        src_l = u_loc[m]
        dst = edge_dst[m]
        order = np.argsort(dst, kind="stable")
        src_l, dst = src_l[order], dst[order]
        g4 = dst // (4 * P)
        segs_g, segs_k = [], []
        for s in range(GS):
            lo = np.searchsorted(g4, s)
            hi = np.searchsorted(g4, s + 1)
            d = dst[lo:hi]
            tilek = (d // P) % 4                       # tile within supertile
            loc = (d % P).astype(np.float32)
            keys = np.full((hi - lo, 4), SENT, np.float32)
            keys[np.arange(hi - lo), tilek] = loc
            segs_g.append(src_l[lo:hi].astype(np.int16))
            segs_k.append(keys)
        C_g.append(segs_g)
        C_k.append(segs_k)
    C_seg_chunks = []
    for s in range(GS):
        mx = max(len(C_g[c][s]) for c in range(W))
        C_seg_chunks.append((mx + P - 1) // P)
    S["C_chunks"] = np.array(C_seg_chunks)          # [GS]
    S["NC"] = int(S["C_chunks"].sum())
    Cg_str, Cl_str = [], []
    for c in range(W):
        g = np.zeros(S["NC"] * P, np.int16)
        k = np.full((S["NC"] * P, 4), SENT, np.float32)
        pos = 0
        for s in range(GS):
            n = len(C_g[c][s])
            g[pos: pos + n] = C_g[c][s]
            k[pos: pos + n] = C_k[c][s]
            pos += C_seg_chunks[s] * P
        Cg_str.append(g)
        # [NC*P, 4] -> [NC, P, 4] -> [P, NC, 4] -> [P, NC*4]
        Cl_str.append(np.ascontiguousarray(
            k.reshape(S["NC"], P, 4).transpose(1, 0, 2).reshape(P, S["NC"] * 4)))

    # ---- Pass D streams: labels by user core, segments = (movie range, user tile) ----
    l_core = lbl_user // USR
    D_u, D_m, D_pos = [], [], []
    for c in range(W):
        m = l_core == c
        idxs = np.nonzero(m)[0]
        ul = (lbl_user[m] - c * USR)
        mv = lbl_movie[m]
        rng = np.minimum(mv // 27008, 2)
        tilev = ul // P
        order = np.lexsort((mv, tilev, rng))
        ul, mv, rng, tilev = ul[order], mv[order], rng[order], tilev[order]
        segs_u, segs_m = [], []
        for r in range(NRNG):
            for t in range(UT):
                mm = (rng == r) & (tilev == t)
                segs_u.append((ul[mm] - t * P).astype(np.float32))
                segs_m.append((mv[mm] - RNG_STARTS[r]).astype(np.int16))
        D_u.append(segs_u)
        D_m.append(segs_m)
        D_pos.append(idxs[order])       # original label index per real stream slot
    # pad segments to max-over-cores
    D_seg_chunks = []
    for s in range(NRNG * UT):
        mx = max(len(D_u[c][s]) for c in range(W))
        D_seg_chunks.append((mx + P - 1) // P)
    S["D_chunks"] = np.array(D_seg_chunks).reshape(NRNG, UT)
    S["ND"] = int(S["D_chunks"].sum())
    Du_str, Dm_str, D_real = [], [], []
    for c in range(W):
        du = np.full(S["ND"] * P, SENT, np.float32)
        dm = np.zeros(S["ND"] * P, np.int16)
        real = np.full(S["ND"] * P, -1, np.int64)
        pos = 0
        k = 0
        for s in range(NRNG * UT):
            n = len(D_u[c][s])
            du[pos: pos + n] = D_u[c][s]
            dm[pos: pos + n] = D_m[c][s]
            real[pos: pos + n] = D_pos[c][k: k + n]
            k += n
            pos += D_seg_chunks[s] * P
        Du_str.append(np.ascontiguousarray(du.reshape(S["ND"], P).T))
        Dm_str.append(dm)
        D_real.append(real)

    iota_rep = np.tile(np.arange(P, dtype=np.float32)[None, :], (P, 4)).astype(npbf16)

    # ---- host-side degree counts (pure index data) ----
    cnt_m_full = np.zeros(NMP, np.float32)
    cnt_m_full[:NM] = np.bincount(edge_dst, minlength=NM)[:NM]

    per_core = []
    for c in range(W):
        sl = cnt_m_full[c * MSL:(c + 1) * MSL].reshape(MT, P)
        mrecip = np.ascontiguousarray((1.0 / np.maximum(sl, 1.0)).T)
        mind = np.ascontiguousarray((sl > 0).astype(np.float32).T)
        cu = np.bincount(u_loc[u_core == c], minlength=USR).astype(np.float32)
        urecip = np.ascontiguousarray(
            (1.0 / np.maximum(cu, 1.0)).reshape(UT, P).T)
        per_core.append({
            "b_loc": Bl_str[c].astype(npbf16),
            "b_gidx": _wrap16(Bg_str[c]),
            "c_loc": Cl_str[c].astype(npbf16),
            "c_gidx": _wrap16(Cg_str[c]),
            "d_uloc": Du_str[c].astype(npbf16),
            "d_midx": _wrap16(Dm_str[c]),
            "iota": iota_rep,
            "mrecip": mrecip,
            "mind": mind,
            "urecip": urecip,
        })
    return S, per_core, D_real


def _gather_groups(n_chunks_list):
    """Split a list of per-segment chunk counts into dma_gather groups of <=GROUP chunks,
    never crossing the segment-list boundary. Returns list of group sizes (in chunks)."""
    total = sum(n_chunks_list)
    groups = []
    rem = total
    while rem > 0:
        g = min(GROUP, rem)
        groups.append(g)
        rem -= g
    return groups


# ---------------- device program ----------------

def build_program(S):
    import os
    UPTO = int(os.environ.get('KUPTO', '9'))
    nc = bacc.Bacc("TRN2", target_bir_lowering=False, debug=False, num_devices=W,
                   num_swdge_queues=4)
    qctr = [0]

    def next_q():
        q = qctr[0] % 4
        qctr[0] += 1
        return q
    NB, NC, ND = S["NB"], S["NC"], S["ND"]
    B_chunks, C_chunks, D_chunks = S["B_chunks"], S["C_chunks"], S["D_chunks"]

    # ---- kernel I/O ----
    featsT = nc.dram_tensor("featsT", [FD, MSL], f32, kind="ExternalInput")
    wm = nc.dram_tensor("wm", [FD, H], f32, kind="ExternalInput")
    u0 = nc.dram_tensor("u0", [H], f32, kind="ExternalInput")
    wnames = ["bm", "wl1_um", "bl1_um", "wr1_um", "wl1_mu", "bl1_mu", "wr1_mu",
              "wl2_um", "bl2_um", "wr2_um", "wl2_mu", "bl2_mu", "wr2_mu"]
    wt = {}
    for n in wnames:
        shape = [H] if n.startswith("b") else [H, H]
        wt[n] = nc.dram_tensor(n, shape, f32, kind="ExternalInput")
    iota_in = nc.dram_tensor("iota", [P, 4 * P], bf16, kind="ExternalInput")
    mrecip_in = nc.dram_tensor("mrecip", [P, MT], f32, kind="ExternalInput")
    mind_in = nc.dram_tensor("mind", [P, MT], f32, kind="ExternalInput")
    urecip_in = nc.dram_tensor("urecip", [P, UT], f32, kind="ExternalInput")
    b_loc = nc.dram_tensor("b_loc", [P, NB * 4], bf16, kind="ExternalInput")
    b_gidx = nc.dram_tensor("b_gidx", [P, NB * 8], i16, kind="ExternalInput")
    c_loc = nc.dram_tensor("c_loc", [P, NC * 4], bf16, kind="ExternalInput")
    c_gidx = nc.dram_tensor("c_gidx", [P, NC * 8], i16, kind="ExternalInput")
    d_uloc = nc.dram_tensor("d_uloc", [P, ND], bf16, kind="ExternalInput")
    d_midx = nc.dram_tensor("d_midx", [P, ND * 8], i16, kind="ExternalInput")
    out = nc.dram_tensor("out", [P, ND], f32, kind="ExternalOutput")

    # ---- internal DRAM ----
    xcat_slice = nc.dram_tensor("xcat_slice", [MSL, 2 * H], bf16)
    xcat_full = nc.dram_tensor("xcat_full", [NMP, 2 * H], bf16, addr_space="Shared")
    mht_stash = nc.dram_tensor("mht_stash", [P, MSL], bf16)
    userh = nc.dram_tensor("userh", [USR, H], bf16)
    usero = nc.dram_tensor("usero", [P, UT * H], bf16)   # tile-major: [p, t*H+h]
    partials = nc.dram_tensor("partials", [NMP, H], bf16)
    parts_rs = nc.dram_tensor("parts_rs", [MSL, H], bf16)
    mo_slice = nc.dram_tensor("mo_slice", [MSL, H], bf16)
    mo_full = nc.dram_tensor("mo_full", [NMP, H], bf16, addr_space="Shared")

    rg = [list(range(W))]

    from contextlib import ExitStack
    with tile.TileContext(nc) as tc, ExitStack() as stack:
        cst = stack.enter_context(tc.tile_pool(name="cst", bufs=1))

        # ---------- constants ----------
        iota_t = cst.tile([P, 4 * P], bf16)
        nc.sync.dma_start(out=iota_t[:], in_=iota_in[:])
        ones_bf = cst.tile([P, 1], bf16)
        nc.vector.memset(ones_bf[:], 1.0)
        ident_bf = cst.tile([P, P], bf16)
        make_identity(nc, ident_bf[:])
        ones_row = cst.tile([1, P], f32)
        nc.vector.memset(ones_row[:], 1.0)
        u0_col = cst.tile([P, 1], f32)
        nc.sync.dma_start(out=u0_col[:], in_=u0[:, None])

        wtile = {}
        for n in wnames:
            if n.startswith("b"):
                t = cst.tile([1, P], f32, tag=f"w_{n}")
                nc.sync.dma_start(out=t[:], in_=wt[n][None, :])
            else:
                t = cst.tile([P, P], f32, tag=f"w_{n}")
                nc.sync.dma_start(out=t[:], in_=wt[n][:])
            wtile[n] = t
        # casts
        w_r = {}
        for n in ["wr1_um", "wl1_mu"]:
            t = cst.tile([P, P], f32r, tag=f"wr_{n}")
            nc.vector.tensor_copy(out=t[:], in_=wtile[n][:])
            w_r[n] = t
        w_bf = {}
        for n in ["wr2_mu", "wr2_um", "wl2_um", "wl2_mu"]:
            t = cst.tile([P, P], bf16, tag=f"wbf_{n}")
            nc.vector.tensor_copy(out=t[:], in_=wtile[n][:])
            w_bf[n] = t
        wm_r = []
        for k in range(4):
            t = cst.tile([P, H], f32r, tag=f"wm_{k}")
            nc.sync.dma_start(out=t[:], in_=wm[k * P:(k + 1) * P, :].bitcast(f32r))
            wm_r.append(t)
        bm_col = cst.tile([P, 1], f32)
        nc.sync.dma_start(out=bm_col[:], in_=wt["bm"][:, None])
        bl1um_col = cst.tile([P, 1], f32)
        nc.sync.dma_start(out=bl1um_col[:], in_=wt["bl1_um"][:, None])

        # v_row = u0 @ Wl1_um   [1,128]; r1_row = u0 @ Wr1_mu
        with tc.tile_pool(name="psc", bufs=2, space="PSUM") as psc:
            vp = psc.tile([1, P], f32, space="PSUM", tag="vrow")
            nc.tensor.matmul(out=vp[:], lhsT=u0_col[:], rhs=wtile["wl1_um"][:], start=True, stop=True)
            v_row = cst.tile([1, P], f32)
            nc.vector.tensor_copy(out=v_row[:], in_=vp[:])
            rp = psc.tile([1, P], f32, space="PSUM", tag="vrow")
            nc.tensor.matmul(out=rp[:], lhsT=u0_col[:], rhs=wtile["wr1_mu"][:], start=True, stop=True)
            b1_row = cst.tile([1, P], f32)
            nc.vector.tensor_tensor(out=b1_row[:], in0=rp[:], in1=wtile["bl1_mu"][:], op=mybir.AluOpType.add)

            def bcast_row(row_ap, tag):
                ps = psc.tile([P, P], f32, space="PSUM", tag="bcast")
                nc.tensor.matmul(out=ps[:], lhsT=ones_row[:], rhs=row_ap, start=True, stop=True)
                t = cst.tile([P, P], f32, tag=tag)
                nc.vector.tensor_copy(out=t[:], in_=ps[:])
                return t

            Vcast = bcast_row(v_row[:], "Vcast")
            B1cast = bcast_row(b1_row[:], "B1cast")
            B2cast = bcast_row(wtile["bl2_mu"][:], "B2cast")
            B3cast = bcast_row(wtile["bl2_um"][:], "B3cast")

        if UPTO >= 2:
            # ---------- Stage 0: movie-side tables ----------
            NCT = (MSL + 511) // 512     # 20 col-tiles (last = 384)
            with tc.tile_pool(name="s0_sb", bufs=1) as s0_sb, \
                 tc.tile_pool(name="s0_mx", bufs=2) as s0_mx, \
                 tc.tile_pool(name="s0_ft", bufs=3) as s0_ft, \
                 tc.tile_pool(name="s0_ps", bufs=1, space="PSUM") as s0_ps, \
                 tc.tile_pool(name="s0_pt", bufs=2, space="PSUM") as s0_pt, \
                 tc.tile_pool(name="s0_stg", bufs=3) as s0_stg:
                p1T = s0_sb.tile([P, MSL], bf16)
                A_fm = s0_sb.tile([P, MSL], bf16)
                mhT = s0_sb.tile([P, MSL], bf16)
                indcols = s0_sb.tile([P, MT], f32)
                nc.sync.dma_start(out=indcols[:], in_=mind_in[:])

                for j in range(NCT):
                    c0 = j * 512
                    cw = min(512, MSL - c0)
                    mxps = s0_ps.tile([P, 512], f32, space="PSUM", tag="mx")
                    for k in range(4):
                        ft = s0_ft.tile([P, 512], f32r, tag="ft")
                        nc.sync.dma_start(out=ft[:, :cw], in_=featsT[k * P:(k + 1) * P, c0:c0 + cw].bitcast(f32r))
                        nc.tensor.matmul(out=mxps[:, :cw], lhsT=wm_r[k][:], rhs=ft[:, :cw],
                                         start=(k == 0), stop=(k == 3))
                    mxt = s0_mx.tile([P, 512], f32r, tag="mxt")
                    nc.vector.tensor_tensor(out=mxt[:, :cw], in0=mxps[:, :cw],
                                            in1=bm_col[:].to_broadcast([P, cw]),
                                            op=mybir.AluOpType.add)
                    p1ps = s0_ps.tile([P, 512], f32, space="PSUM", tag="p1")
                    nc.tensor.matmul(out=p1ps[:, :cw], lhsT=w_r["wl1_mu"][:], rhs=mxt[:, :cw],
                                     start=True, stop=True)
                    nc.scalar.copy(out=p1T[:, c0:c0 + cw], in_=p1ps[:, :cw])
                    aps = s0_ps.tile([P, 512], f32, space="PSUM", tag="A")
                    nc.tensor.matmul(out=aps[:, :cw], lhsT=w_r["wr1_um"][:], rhs=mxt[:, :cw],
                                     start=True, stop=True)
                    nc.vector.tensor_tensor(out=A_fm[:, c0:c0 + cw], in0=aps[:, :cw],
                                            in1=bl1um_col[:].to_broadcast([P, cw]),
                                            op=mybir.AluOpType.add)

                # per 128-tile: movie_h row-major then back to feature-major
                for t in range(MT):
                    c0 = t * P
                    tp = s0_pt.tile([P, P], bf16, space="PSUM", tag="tp")
                    nc.tensor.transpose(out=tp[:], in_=A_fm[:, c0:c0 + P], identity=ident_bf[:])
                    term = s0_stg.tile([P, P], f32, tag="term")
                    nc.vector.tensor_tensor(out=term[:], in0=Vcast[:],
                                            in1=indcols[:, t:t + 1].to_broadcast([P, P]),
                                            op=mybir.AluOpType.mult)
                    mhrow = s0_stg.tile([P, P], bf16, tag="mhrow")
                    nc.vector.tensor_tensor(out=mhrow[:], in0=tp[:], in1=term[:],
                                            op=mybir.AluOpType.add)
                    nc.vector.tensor_scalar_max(out=mhrow[:], in0=mhrow[:], scalar1=0.0)
                    tp2 = s0_pt.tile([P, P], bf16, space="PSUM", tag="tp2")
                    nc.tensor.transpose(out=tp2[:], in_=mhrow[:], identity=ident_bf[:])
                    nc.scalar.copy(out=mhT[:, c0:c0 + P], in_=tp2[:])
                nc.sync.dma_start(out=mht_stash[:], in_=mhT[:])

                # p2T = Wl2_mu.T @ mhT  (bf16)
                p2T = s0_sb.tile([P, MSL], bf16)
                for j in range(NCT):
                    c0 = j * 512
                    cw = min(512, MSL - c0)
                    ps = s0_ps.tile([P, 512], f32, space="PSUM", tag="p2")
                    nc.tensor.matmul(out=ps[:, :cw], lhsT=w_bf["wl2_mu"][:], rhs=mhT[:, c0:c0 + cw],
                                     start=True, stop=True)
                    nc.scalar.copy(out=p2T[:, c0:c0 + cw], in_=ps[:, :cw])

                # transpose to row-major X_cat slice and store
                for t in range(MT):
                    c0 = t * P
                    stg = s0_stg.tile([P, 2 * H], bf16, tag="xrow")
                    tp = s0_pt.tile([P, P], bf16, space="PSUM", tag="tp")
                    nc.tensor.transpose(out=tp[:], in_=p1T[:, c0:c0 + P], identity=ident_bf[:])
                    nc.scalar.copy(out=stg[:, 0:H], in_=tp[:])
                    tp2 = s0_pt.tile([P, P], bf16, space="PSUM", tag="tp2")
                    nc.tensor.transpose(out=tp2[:], in_=p2T[:, c0:c0 + P], identity=ident_bf[:])
                    nc.scalar.copy(out=stg[:, H:2 * H], in_=tp2[:])
                    nc.sync.dma_start(out=xcat_slice[c0:c0 + P, :], in_=stg[:])

            nc.gpsimd.collective_compute(
                "AllGather", mybir.AluOpType.bypass, replica_groups=rg,
                ins=[xcat_slice[:].opt()], outs=[xcat_full[:].opt()])

        if UPTO >= 3:
            # ---------- Pass B: user-side fused aggregation ----------
            ACC = 256  # [p1sum 128 | p2sum 128]; counts come from host
            with tc.tile_pool(name="pb_sb", bufs=1) as pb_sb, \
                 tc.tile_pool(name="pb_s4", bufs=8) as pb_s4, \
                 tc.tile_pool(name="pb_g", bufs=5) as pb_g, \
                 tc.tile_pool(name="pb_gi", bufs=5) as pb_gi, \
                 tc.tile_pool(name="pb_ps", bufs=1, space="PSUM") as pb_ps, \
                 tc.tile_pool(name="pb_pt", bufs=3, space="PSUM") as pb_pt, \
                 tc.tile_pool(name="pb_stg", bufs=8) as pb_stg:
                bloc_t = pb_sb.tile([P, NB * 4], bf16)
                nc.sync.dma_start(out=bloc_t[:], in_=b_loc[:])
                accB = pb_sb.tile([P, UT * ACC], bf16)
                nc.vector.memset(accB[:], 0.0)
                recipv = pb_sb.tile([P, UT], f32)
                nc.sync.dma_start(out=recipv[:], in_=urecip_in[:])

                def b_epilogue(t):
                    # user_h / user_o for tile t (accB final after last range)
                    a0 = t * ACC
                    rc = recipv[:, t:t + 1]
                    uh = pb_stg.tile([P, H], bf16, tag="uh")
                    nc.vector.tensor_tensor(out=uh[:], in0=accB[:, a0:a0 + H],
                                            in1=rc.to_broadcast([P, H]), op=mybir.AluOpType.mult)
                    nc.vector.tensor_tensor(out=uh[:], in0=uh[:], in1=B1cast[:],
                                            op=mybir.AluOpType.add)
                    nc.vector.tensor_scalar_max(out=uh[:], in0=uh[:], scalar1=0.0)
                    tp = pb_pt.tile([P, P], bf16, space="PSUM", tag="ep")
                    nc.tensor.transpose(out=tp[:], in_=uh[:], identity=ident_bf[:])
                    uht = pb_stg.tile([P, P], bf16, tag="uhts")
                    nc.scalar.copy(out=uht[:], in_=tp[:])
                    # Pass C gather table = user_h @ Wl2_um (pre-multiplied; linearity)
                    t2ps = pb_pt.tile([P, P], f32, space="PSUM", tag="ep")
                    nc.tensor.matmul(out=t2ps[:], lhsT=uht[:], rhs=w_bf["wl2_um"][:],
                                     start=True, stop=True)
                    uh2 = pb_stg.tile([P, P], bf16, tag="uh2")
                    nc.scalar.copy(out=uh2[:], in_=t2ps[:])
                    nc.sync.dma_start(out=userh[t * P:(t + 1) * P, :], in_=uh2[:])
                    # user_o = p2sum*recip + B2cast + uh @ Wr2_mu
                    rps = pb_pt.tile([P, P], f32, space="PSUM", tag="ep")
                    nc.tensor.matmul(out=rps[:], lhsT=uht[:], rhs=w_bf["wr2_mu"][:],
                                     start=True, stop=True)
                    uo = pb_stg.tile([P, H], f32, tag="uo")
                    nc.vector.tensor_tensor(out=uo[:], in0=accB[:, a0 + H:a0 + 2 * H],
                                            in1=rc.to_broadcast([P, H]), op=mybir.AluOpType.mult)
                    nc.vector.tensor_tensor(out=uo[:], in0=uo[:], in1=B2cast[:],
                                            op=mybir.AluOpType.add)
                    uo_bf = pb_stg.tile([P, H], bf16, tag="uobf")
                    nc.vector.tensor_tensor(out=uo_bf[:], in0=uo[:], in1=rps[:],
                                            op=mybir.AluOpType.add)
                    nc.sync.dma_start(out=usero[:, t * H:(t + 1) * H], in_=uo_bf[:])

                pos = 0          # global chunk position (stream)
                for r in range(NRNG):
                    table = xcat_full[RNG_STARTS[r]:RNG_ENDS[r], :]
                    sub_chunks = int(B_chunks[r].sum())
                    # gather groups for this sub-pass
                    gpos = 0
                    gbufs = []
                    while gpos < sub_chunks:
                        gn = min(GROUP, sub_chunks - gpos)
                        gb = pb_g.tile([P, GROUP * 2 * H], bf16, tag="gbuf")
                        gi = pb_gi.tile([P, GROUP * 8], i16, tag="gidx")
                        col0 = (pos + gpos) * 8
                        nc.sync.dma_start(out=gi[:, :gn * 8], in_=b_gidx[:, col0: col0 + gn * 8])
                        nc.gpsimd.dma_gather(
                            out_ap=gb[:, :gn * 2 * H].rearrange("p (c n) -> p c n", c=gn),
                            in_ap=table,
                            idxs_ap=gi[:, :gn * 8],
                            num_idxs=gn * P,
                            num_idxs_reg=gn * P,
                            elem_size=2 * H,
                            queue_num=next_q(),
                        )
                        gbufs.append((gpos, gn, gb))
                        gpos += gn

                    def get_slot(sub_pos):
                        for g0, gn, gb in gbufs:
                            if g0 <= sub_pos < g0 + gn:
                                return gb, sub_pos - g0
                        raise AssertionError

                    sub_pos = 0
                    for t4 in range(UT // 4):
                        n = int(B_chunks[r][t4])
                        if n == 0:
                            if r == NRNG - 1:
                                for k in range(4):
                                    b_epilogue(4 * t4 + k)
                            continue
                        ps4 = []
                        for k in range(4):
                            pst = pb_ps.tile([P, 2 * H], f32, space="PSUM", tag=f"bps{k}")
                            ps4.append(pst[:])
                        for ch in range(n):
                            cc = pos + sub_pos + ch
                            s4 = pb_s4.tile([P, 4 * P], bf16, tag="s4")
                            nc.vector.tensor_tensor(
                                out=s4[:].rearrange("p (k n) -> p k n", k=4),
                                in0=iota_t[:].rearrange("p (k n) -> p k n", k=4),
                                in1=bloc_t[:, 4 * cc: 4 * cc + 4][:, :, None].to_broadcast([P, 4, P]),
                                op=mybir.AluOpType.is_equal,
                            )
                            gb, slot = get_slot(sub_pos + ch)
                            for k in range(4):
                                nc.tensor.matmul(
                                    out=ps4[k],
                                    lhsT=s4[:, k * P:(k + 1) * P],
                                    rhs=gb[:, slot * 2 * H:(slot + 1) * 2 * H],
                                    start=(ch == 0), stop=(ch == n - 1),
                                )
                        for k in range(4):
                            t = 4 * t4 + k
                            a0 = t * ACC
                            nc.vector.tensor_tensor(out=accB[:, a0:a0 + 2 * H], in0=ps4[k],
                                                    in1=accB[:, a0:a0 + 2 * H], op=mybir.AluOpType.add)
                            if r == NRNG - 1:
                                b_epilogue(t)
                        sub_pos += n
                    pos += sub_chunks

        if UPTO >= 4:
            # ---------- Pass C: movie-side aggregation of user_h ----------
            GS = GMT // 4
            with tc.tile_pool(name="pc_sb", bufs=1) as pc_sb, \
                 tc.tile_pool(name="pc_s4", bufs=8) as pc_s4, \
                 tc.tile_pool(name="pc_g", bufs=5) as pc_g, \
                 tc.tile_pool(name="pc_gi", bufs=5) as pc_gi, \
                 tc.tile_pool(name="pc_ps", bufs=2, space="PSUM") as pc_ps, \
                 tc.tile_pool(name="pc_stg", bufs=8) as pc_stg:
                cloc_t = pc_sb.tile([P, NC * 4], bf16)
                nc.sync.dma_start(out=cloc_t[:], in_=c_loc[:])

                gpos = 0
                gbufs = []
                while gpos < NC:
                    gn = min(GROUP, NC - gpos)
                    gb = pc_g.tile([P, GROUP * H], bf16, tag="gbuf")
                    gi = pc_gi.tile([P, GROUP * 8], i16, tag="gidx")
                    nc.sync.dma_start(out=gi[:, :gn * 8], in_=c_gidx[:, gpos * 8: (gpos + gn) * 8])
                    nc.gpsimd.dma_gather(
                        out_ap=gb[:, :gn * H].rearrange("p (c n) -> p c n", c=gn),
                        in_ap=userh[:],
                        idxs_ap=gi[:, :gn * 8],
                        num_idxs=gn * P,
                        num_idxs_reg=gn * P,
                        elem_size=H,
                        queue_num=next_q(),
                    )
                    gbufs.append((gpos, gn, gb))
                    gpos += gn

                def get_slotC(p_):
                    for g0, gn, gb in gbufs:
                        if g0 <= p_ < g0 + gn:
                            return gb, p_ - g0
                    raise AssertionError

                pos = 0
                for s in range(GS):
                    n = int(C_chunks[s])
                    if n == 0:
                        for k in range(4):
                            stg = pc_stg.tile([P, H], bf16, tag="pstg")
                            nc.vector.memset(stg[:], 0.0)
                            g = 4 * s + k
                            nc.sync.dma_start(out=partials[g * P:(g + 1) * P, :], in_=stg[:])
                        continue
                    ps4 = []
                    for k in range(4):
                        pst = pc_ps.tile([P, H], f32, space="PSUM", tag=f"ps{k}")
                        ps4.append(pst[:])
                    for ch in range(n):
                        cc = pos + ch
                        s4 = pc_s4.tile([P, 4 * P], bf16, tag="s4")
                        nc.vector.tensor_tensor(
                            out=s4[:].rearrange("p (k n) -> p k n", k=4),
                            in0=iota_t[:].rearrange("p (k n) -> p k n", k=4),
                            in1=cloc_t[:, 4 * cc: 4 * cc + 4][:, :, None].to_broadcast([P, 4, P]),
                            op=mybir.AluOpType.is_equal,
                        )
                        gb, slot = get_slotC(cc)
                        for k in range(4):
                            nc.tensor.matmul(
                                out=ps4[k],
                                lhsT=s4[:, k * P:(k + 1) * P],
                                rhs=gb[:, slot * H:(slot + 1) * H],
                                start=(ch == 0), stop=(ch == n - 1),
                            )
                    for k in range(4):
                        stg = pc_stg.tile([P, H], bf16, tag="pstg")
                        nc.scalar.copy(out=stg[:], in_=ps4[k])
                        g = 4 * s + k
                        nc.sync.dma_start(out=partials[g * P:(g + 1) * P, :], in_=stg[:])
                    pos += n

        if UPTO >= 5:
            # ---------- movie_o (root terms prestashed to overlap the RS) ----------
            with tc.tile_pool(name="mo_sb", bufs=1) as mo_sb, \
                 tc.tile_pool(name="mo_in", bufs=6) as mo_in, \
                 tc.tile_pool(name="mo_ps", bufs=2, space="PSUM") as mo_ps, \
                 tc.tile_pool(name="mo_stg", bufs=4) as mo_stg:
                recipm = mo_sb.tile([P, MT], f32)
                nc.sync.dma_start(out=recipm[:], in_=mrecip_in[:])
                roots = mo_sb.tile([P, MT * H], bf16)
                for t in range(MT):
                    mh = mo_in.tile([P, P], bf16, tag="mh")
                    nc.sync.dma_start(out=mh[:], in_=mht_stash[:, t * P:(t + 1) * P])
                    rps = mo_ps.tile([P, P], f32, space="PSUM", tag="mroot")
                    nc.tensor.matmul(out=rps[:], lhsT=mh[:], rhs=w_bf["wr2_um"][:],
                                     start=True, stop=True)
                    nc.vector.tensor_tensor(out=roots[:, t * H:(t + 1) * H], in0=rps[:],
                                            in1=B3cast[:], op=mybir.AluOpType.add)

                nc.gpsimd.collective_compute(
                    "ReduceScatter", mybir.AluOpType.add, replica_groups=rg,
                    ins=[partials[:].opt()], outs=[parts_rs[:].opt()])

                for t in range(MT):
                    pin = mo_in.tile([P, H], bf16, tag="pin")
                    nc.sync.dma_start(out=pin[:], in_=parts_rs[t * P:(t + 1) * P, :])
                    mo_t = mo_stg.tile([P, H], f32, tag="mo1")
                    nc.vector.tensor_tensor(out=mo_t[:], in0=pin[:],
                                            in1=recipm[:, t:t + 1].to_broadcast([P, H]),
                                            op=mybir.AluOpType.mult)
                    mo_bf = mo_stg.tile([P, H], bf16, tag="mo2")
                    nc.vector.tensor_tensor(out=mo_bf[:], in0=mo_t[:],
                                            in1=roots[:, t * H:(t + 1) * H],
                                            op=mybir.AluOpType.add)
                    nc.sync.dma_start(out=mo_slice[t * P:(t + 1) * P, :], in_=mo_bf[:])

            nc.gpsimd.collective_compute(
                "AllGather", mybir.AluOpType.bypass, replica_groups=rg,
                ins=[mo_slice[:].opt()], outs=[mo_full[:].opt()])

        if UPTO >= 6:
            # ---------- Pass D: label dots (user side via one-hot gather) ----------
            with tc.tile_pool(name="pd_sb", bufs=1) as pd_sb, \
                 tc.tile_pool(name="pd_g", bufs=8) as pd_g, \
                 tc.tile_pool(name="pd_gi", bufs=8) as pd_gi, \
                 tc.tile_pool(name="pd_pt", bufs=4, space="PSUM") as pd_pt, \
                 tc.tile_pool(name="pd_stg", bufs=8) as pd_stg:
                outstrip = pd_sb.tile([P, ND], f32)
                dloc_t = pd_sb.tile([P, ND], bf16)
                nc.sync.dma_start(out=dloc_t[:], in_=d_uloc[:])
                uo_all = pd_sb.tile([P, UT * H], bf16)
                nc.sync.dma_start(out=uo_all[:], in_=usero[:])
                pos = 0
                for r in range(NRNG):
                    sub_chunks = int(D_chunks[r].sum())
                    table = mo_full[RNG_STARTS[r]:RNG_ENDS[r], :]
                    gpos = 0
                    gbufs = []
                    while gpos < sub_chunks:
                        gn = min(GROUP, sub_chunks - gpos)
                        gm = pd_g.tile([P, GROUP * H], bf16, tag="gm")
                        gim = pd_gi.tile([P, GROUP * 8], i16, tag="gim")
                        col0 = (pos + gpos) * 8
                        nc.sync.dma_start(out=gim[:, :gn * 8], in_=d_midx[:, col0: col0 + gn * 8])
                        nc.gpsimd.dma_gather(
                            out_ap=gm[:, :gn * H].rearrange("p (c n) -> p c n", c=gn),
                            in_ap=table, idxs_ap=gim[:, :gn * 8],
                            num_idxs=gn * P, num_idxs_reg=gn * P, elem_size=H,
                            queue_num=next_q())
                        gbufs.append((gpos, gn, gm))
                        gpos += gn

                    def get_slotD(p_, _gb=gbufs):
                        for g0, gn, gb in _gb:
                            if g0 <= p_ < g0 + gn:
                                return gb, p_ - g0
                        raise AssertionError

                    sub_pos = 0
                    for t in range(UT):
                        n = int(D_chunks[r][t])
                        if n == 0:
                            continue
                        uo_t = uo_all[:, t * H:(t + 1) * H]
                        for ch in range(n):
                            cc = pos + sub_pos + ch
                            s1 = pd_stg.tile([P, P], bf16, tag="s1")
                            nc.vector.tensor_tensor(
                                out=s1[:], in0=iota_t[:, 0:P],
                                in1=dloc_t[:, cc:cc + 1].to_broadcast([P, P]),
                                op=mybir.AluOpType.is_equal)
                            tp = pd_pt.tile([P, P], bf16, space="PSUM", tag="tp")
                            nc.tensor.transpose(out=tp[:], in_=s1[:], identity=ident_bf[:])
                            oh = pd_stg.tile([P, P], bf16, tag="oh")
                            nc.scalar.copy(out=oh[:], in_=tp[:])
                            ups = pd_pt.tile([P, H], f32, space="PSUM", tag="ups")
                            nc.tensor.matmul(out=ups[:], lhsT=oh[:], rhs=uo_t,
                                             start=True, stop=True)
                            gb, slot = get_slotD(sub_pos + ch)
                            pr = pd_stg.tile([P, H], f32, tag="pr")
                            nc.vector.tensor_tensor(out=pr[:], in0=ups[:],
                                                    in1=gb[:, slot * H:(slot + 1) * H],
                                                    op=mybir.AluOpType.mult)
                            nc.vector.tensor_reduce(
                                out=outstrip[:, cc:cc + 1], in_=pr[:],
                                axis=mybir.AxisListType.X, op=mybir.AluOpType.add)
                        sub_pos += n
                    pos += sub_chunks
                nc.sync.dma_start(out=out[:], in_=outstrip[:])
        else:
            with tc.tile_pool(name="dummy", bufs=1) as dp:
                z = dp.tile([P, ND], f32)
                nc.vector.memset(z[:], 0.0)
                nc.sync.dma_start(out=out[:], in_=z[:])

    nc.compile()
    return nc


# ---------------- entry point ----------------

_CACHE = {}
TRACE = False
LAST_EXEC_NS = None
LAST_RESULTS = None


def kernel(movie_feats, user_init, edge_src, edge_dst, lbl_user, lbl_movie, n_users,
           Wm, bm,
           Wl1_um, bl1_um, Wr1_um, Wl1_mu, bl1_mu, Wr1_mu,
           Wl2_um, bl2_um, Wr2_um, Wl2_mu, bl2_mu, Wr2_mu):
    movie_feats = np.asarray(movie_feats, dtype=np.float32)
    S, per_core, D_real = preprocess(edge_src, edge_dst, lbl_user, lbl_movie)

    key = (S["NB"], S["NC"], S["ND"],
           S["B_chunks"].tobytes(), S["C_chunks"].tobytes(), S["D_chunks"].tobytes())
    if key in _CACHE:
        nc = _CACHE[key]
    else:
        nc = build_program(S)
        _CACHE[key] = nc

    featsT = np.zeros((FD, NMP), np.float32)
    featsT[:, :NM] = movie_feats.T

    weights = {
        "wm": np.asarray(Wm, np.float32), "u0": np.asarray(user_init, np.float32),
        "bm": np.asarray(bm, np.float32),
        "wl1_um": np.asarray(Wl1_um, np.float32), "bl1_um": np.asarray(bl1_um, np.float32),
        "wr1_um": np.asarray(Wr1_um, np.float32),
        "wl1_mu": np.asarray(Wl1_mu, np.float32), "bl1_mu": np.asarray(bl1_mu, np.float32),
        "wr1_mu": np.asarray(Wr1_mu, np.float32),
        "wl2_um": np.asarray(Wl2_um, np.float32), "bl2_um": np.asarray(bl2_um, np.float32),
        "wr2_um": np.asarray(Wr2_um, np.float32),
        "wl2_mu": np.asarray(Wl2_mu, np.float32), "bl2_mu": np.asarray(bl2_mu, np.float32),
        "wr2_mu": np.asarray(Wr2_mu, np.float32),
    }

    in_maps = []
    for c in range(W):
        m = {"featsT": np.ascontiguousarray(featsT[:, c * MSL:(c + 1) * MSL])}
        m.update(weights)
        pc = per_core[c]
        m.update({
            "iota": pc["iota"],
            "b_loc": pc["b_loc"], "b_gidx": pc["b_gidx"],
            "c_loc": pc["c_loc"], "c_gidx": pc["c_gidx"],
            "d_uloc": pc["d_uloc"], "d_midx": pc["d_midx"],
            "mrecip": pc["mrecip"], "mind": pc["mind"], "urecip": pc["urecip"],
        })
        in_maps.append(m)

    global LAST_EXEC_NS, LAST_RESULTS
    res = run_bass_kernel_spmd(nc, in_maps, core_ids=list(range(W)), trace=TRACE)
    LAST_EXEC_NS = res.exec_time_ns
    LAST_RESULTS = res

    EL = len(np.asarray(lbl_user))
    out_full = np.zeros(EL, np.float32)
    for c in range(W):
        vals = res.results[c]["out"].T.reshape(-1)       # stream order
        real = D_real[c]
        mask = real >= 0
        out_full[real[mask]] = vals[mask]
    return out_full

